# revision 1
# baseline (speedup 1.0000x reference)
"""Trainium2 Bass kernel for nn_MixtureOfRanksLayer (moe_routing).

Strategy: data-parallel over the token axis (N=4096 -> 512 tokens/core on 8
cores), all 8 experts evaluated densely per core (weighted by the top-2
routing mask, which matches the reference math exactly).  No collectives:
the full output is just the concatenation of per-core token slices.

Per-core pipeline (tok = 512 on the matmul free dim throughout):
  xT   = x.T                         (PE transpose, 16x4 128x128 tiles)
  T1T  = U1cat.T @ xT                [er=512, tok]   K=2048, fp32r
  lg   = x @ gate_w.T + gate_b       [tok, 8]        via xT as lhsT
  w    = top2-renormalized weights   (masked-max + sigmoid(l1-l2), exact)
  per expert pair (2c, 2c+1):
    h0/h1 = relu(v1.T @ T1T + b1)    row-tiled K=64 matmul pairs, bf16
                                     (bf16 => separate LDWEIGHTS => the
                                     pair runs concurrently; fp32r's fused
                                     weight-load cannot overlap)
    T2T  += u2.T-chunks @ hT         col-tiled concurrent pairs, bf16
  T2T *= w (broadcast via SEL matmul), OUT = T2T.T @ V2cat + w.T @ b2
  relu evacuations interleave ScalarE/VectorE per iteration (never in
  blocks: blocky splits collapse the two PSUM drain lanes to average-rate).

Walrus limits each compute-engine instruction to ONE sync wait at the ISA
level; building on bacc.Bacc and calling nc.compile() legalizes the
multi-wait instructions Tile emits.
"""

from contextlib import ExitStack

import numpy as np

import concourse.bass as bass
import concourse.bacc as bacc
import concourse.mybir as mybir
import concourse.tile as tile
from concourse.tile_rust import add_dep_helper

dt = mybir.dt
AF = mybir.ActivationFunctionType
ALU = mybir.AluOpType
AX = mybir.AxisListType

# Model dims (hardcoded for this problem)
E, D, H, R = 8, 2048, 8192, 64
N_TOK = 4096
NCORES = 8

FULL_CFG = dict(E=E, D=D, H=H, R=R, NT=N_TOK // NCORES)

# fraction of relu/evac chunks handled by ScalarE (rest on VectorE)
ACT_OF_16 = 10


def build(cfg, rep=1):
    """Build the single-core Bass module (SPMD: same NEFF on all cores)."""
    E, D, H, R, NT = cfg["E"], cfg["D"], cfg["H"], cfg["R"], cfg["NT"]
    TOKC = NT // 128          # token chunks of 128
    DC = D // 128             # contraction chunks over d_model
    HC = H // 128             # hidden chunks of 128
    ER = E * R                # stacked expert-rank axis (512)
    ERC = E // 2              # expert pairs
    DD = D // 512             # output free-dim chunks
    f32 = dt.float32
    f32r = dt.float32r
    bf16 = dt.bfloat16

    nc = bacc.Bacc("TRN2", debug=False)

    x_d = nc.dram_tensor("x", [NT, D], f32, kind="ExternalInput").ap()
    u1c_d = nc.dram_tensor("u1c", [D, ER], f32r, kind="ExternalInput").ap()
    v1_d = nc.dram_tensor("v1", [E, R, H], bf16, kind="ExternalInput").ap()
    b1r_d = nc.dram_tensor("b1r", [128, E * HC], f32, kind="ExternalInput").ap()
    u2r_d = nc.dram_tensor("u2r", [E, 128, HC, R], bf16, kind="ExternalInput").ap()
    v2c_d = nc.dram_tensor("v2c", [ER, D], f32r, kind="ExternalInput").ap()
    b2_d = nc.dram_tensor("b2", [E, D], f32r, kind="ExternalInput").ap()
    gw_d = nc.dram_tensor("gw", [E, D], f32, kind="ExternalInput").ap()
    gb_d = nc.dram_tensor("gb", [1, E], f32r, kind="ExternalInput").ap()
    onesr_d = nc.dram_tensor("onesr", [1, 128], f32r, kind="ExternalInput").ap()
    ident_d = nc.dram_tensor("ident", [128, 128], f32, kind="ExternalInput").ap()
    sel_d = nc.dram_tensor("sel", [E, ERC, 128], f32r, kind="ExternalInput").ap()
    out_d = nc.dram_tensor("out", [NT, D], f32, kind="ExternalOutput").ap()

    with ExitStack() as ctx:
        tc = ctx.enter_context(tile.TileContext(nc))

        const = ctx.enter_context(tc.tile_pool(name="const", bufs=1))
        persist = ctx.enter_context(tc.tile_pool(name="persist", bufs=1))

        ident = const.tile([128, 128], f32, tag="ident")
        nc.sync.dma_start(ident, ident_d)
        sel_sb = const.tile([E, ERC, 128], f32r, tag="sel")
        nc.sync.dma_start(sel_sb, sel_d)
        ones_sb = const.tile([1, 128], f32r, tag="ones")
        nc.sync.dma_start(ones_sb, onesr_d)
        gb_sb = const.tile([1, E], f32r, tag="gb")
        nc.sync.dma_start(gb_sb, gb_d)
        b2_sb = const.tile([E, D], f32r, tag="b2")
        nc.sync.dma_start(b2_sb, b2_d)
        b1r_sb = const.tile([128, E * HC], f32, tag="b1r")
        nc.sync.dma_start(b1r_sb, b1r_d)
        gw_sb = const.tile([E, D], f32, tag="gw")
        nc.sync.dma_start(gw_sb, gw_d)

        gwT = persist.tile([128, DC, E], f32r, tag="gwT")
        T1Tp = [persist.tile([128, NT], bf16, tag=f"t1t{c}", name=f"t1t{c}")
                for c in range(ERC)]
        wT = persist.tile([E, NT], f32r, tag="wT")
        Wbc = [persist.tile([128, NT], f32, tag=f"wbc{c}", name=f"wbc{c}")
               for c in range(ERC)]
        T2Ts = [persist.tile([128, NT], f32r, tag=f"t2t{c}", name=f"t2t{c}")
                for c in range(ERC)]
        V2sb = [persist.tile([128, D], f32r, tag=f"v2{c}", name=f"v2{c}")
                for c in range(ERC)]


        # ---------------- Phase 1: xT, T1T, gating ----------------
        with ExitStack() as s1:
            p1 = s1.enter_context(tc.tile_pool(name="p1", bufs=1))
            sm = s1.enter_context(tc.tile_pool(name="sm", bufs=2))
            ps_tp = s1.enter_context(tc.tile_pool(name="ps_tp", bufs=2, space="PSUM"))
            ps_t1 = s1.enter_context(tc.tile_pool(name="ps_t1", bufs=2, space="PSUM"))
            ps_lg = s1.enter_context(tc.tile_pool(name="ps_lg", bufs=1, space="PSUM"))
            ps_wb = s1.enter_context(tc.tile_pool(name="ps_wb", bufs=2, space="PSUM"))

            x_sb = p1.tile([128, TOKC, D], f32, tag="x")
            for t in range(TOKC):
                nc.sync.dma_start(x_sb[:, t, :], x_d[t * 128:(t + 1) * 128, :])
            U1sb = p1.tile([128, DC, ER], f32r, tag="u1")
            nc.sync.dma_start(U1sb, u1c_d.rearrange("(dc p) er -> p dc er", p=128))
            xT = p1.tile([128, DC, NT], f32r, tag="xT")


            # gate_w.T tiles via PE transpose (evac: DVE)
            for dc in range(DC):
                pst = ps_tp.tile([128, 128], f32, tag="tp")
                nc.tensor.transpose(pst[:, 0:E], gw_sb[:, dc * 128:(dc + 1) * 128],
                                    ident[0:E, 0:E])
                nc.vector.tensor_copy(gwT[:, dc, :], pst[:, 0:E])

            # x.T tiles via PE transpose (evac: DVE)
            for dc in range(DC):
                for t in range(TOKC):
                    pst = ps_tp.tile([128, 128], f32, tag="tp")
                    nc.tensor.transpose(pst, x_sb[:, t, dc * 128:(dc + 1) * 128], ident)
                    dst = xT[:, dc, t * 128:(t + 1) * 128]
                    if (dc * TOKC + t) % 2 == 0:
                        nc.scalar.copy(dst, pst)
                    else:
                        nc.vector.tensor_copy(dst, pst)


            # T1T = U1.T @ xT   [er, tok], accumulated over DC (evac: ACT)
            for c in range(ERC):
                pt = ps_t1.tile([128, NT], f32, tag="t1")
                for dc in range(DC):
                    nc.tensor.matmul(pt,
                                     lhsT=U1sb[:, dc, c * 128:(c + 1) * 128],
                                     rhs=xT[:, dc, :],
                                     start=(dc == 0), stop=(dc == DC - 1))
                if c % 2 == 0:
                    nc.scalar.copy(T1Tp[c], pt)
                else:
                    nc.vector.tensor_copy(T1Tp[c], pt)


            # logits, top-2 renormalized weights
            for t in range(TOKC):
                pl = ps_lg.tile([128, E], f32, tag="lg")
                for dc in range(DC):
                    nc.tensor.matmul(pl,
                                     lhsT=xT[:, dc, t * 128:(t + 1) * 128],
                                     rhs=gwT[:, dc, :],
                                     start=(dc == 0), stop=False)
                nc.tensor.matmul(pl, lhsT=ones_sb, rhs=gb_sb,
                                 start=False, stop=True)
                lg = sm.tile([128, E], f32, tag="lg_sb")
                nc.vector.tensor_copy(lg, pl)
                l1 = sm.tile([128, 1], f32, tag="l1")
                nc.vector.reduce_max(out=l1, in_=lg, axis=AX.X)
                m1t = sm.tile([128, E], f32, tag="m1t")
                nc.vector.tensor_scalar(m1t, lg, l1, None, op0=ALU.is_equal)
                lm = sm.tile([128, E], f32, tag="lm")
                nc.vector.tensor_scalar(lm, m1t, -1e30, None, op0=ALU.mult)
                nc.vector.tensor_add(lm, lm, lg)
                l2 = sm.tile([128, 1], f32, tag="l2")
                nc.vector.reduce_max(out=l2, in_=lm, axis=AX.X)
                m2t = sm.tile([128, E], f32, tag="m2t")
                nc.vector.tensor_scalar(m2t, lm, l2, None, op0=ALU.is_equal)
                dif = sm.tile([128, 1], f32, tag="dif")
                nc.vector.tensor_sub(dif, l1, l2)
                s1v = sm.tile([128, 1], f32, tag="s1v")
                nc.scalar.activation(s1v, dif, AF.Sigmoid)
                s0v = sm.tile([128, 1], f32, tag="s0v")
                nc.scalar.activation(s0v, dif, AF.Sigmoid, scale=-1.0)
                wa = sm.tile([128, E], f32, tag="wa")
                nc.vector.tensor_scalar(wa, m1t, s1v, None, op0=ALU.mult)
                wb_ = sm.tile([128, E], f32, tag="wb_")
                nc.vector.tensor_scalar(wb_, m2t, s0v, None, op0=ALU.mult)
                w_sb = sm.tile([128, E], f32, tag="w_sb")
                nc.vector.tensor_add(w_sb, wa, wb_)
                pw = ps_tp.tile([128, 128], f32, tag="tp")
                nc.tensor.transpose(pw[0:E, :], w_sb, ident)
                nc.vector.tensor_copy(wT[:, t * 128:(t + 1) * 128], pw[0:E, :])


            # broadcast per-expert weights across partitions: SEL.T @ wT
            for c in range(ERC):
                pb = ps_wb.tile([128, NT], f32, tag="wb")
                nc.tensor.matmul(pb, lhsT=sel_sb[:, c, :], rhs=wT,
                                 start=True, stop=True)
                nc.vector.tensor_copy(Wbc[c], pb)

            # engine-tick observers: bring PE's view of the ACT/DVE clocks
            # current so later pool-alloc deps reduce to single waits
            obs_a = persist.tile([1, 1], f32, tag="obs_a")
            nc.scalar.copy(obs_a, b1r_sb[0:1, 0:1])

        # ---------------- Phase 2: experts ----------------
        p2v = ctx.enter_context(tc.tile_pool(name="p2v", bufs=2))
        p2u = ctx.enter_context(tc.tile_pool(name="p2u", bufs=2))
        p2h = ctx.enter_context(tc.tile_pool(name="p2h", bufs=8))
        s2 = ExitStack()
        ps_h0 = s2.enter_context(tc.tile_pool(name="ps_h0", bufs=3, space="PSUM"))
        ps_h1 = s2.enter_context(tc.tile_pool(name="ps_h1", bufs=3, space="PSUM"))
        ps_t2 = s2.enter_context(tc.tile_pool(name="ps_t2", bufs=1, space="PSUM"))

        def relu_evac(dst, src, bias_ap, on_act):
            if on_act:
                nc.scalar.activation(dst, src, AF.Relu, bias=bias_ap)
            else:
                nc.vector.tensor_scalar(dst, src, bias_ap, 0.0,
                                        op0=ALU.add, op1=ALU.max)

        for c0 in range(ERC * rep):
            c = c0 % ERC
            e0, e1 = 2 * c, 2 * c + 1
            v1p = p2v.tile([128, H], bf16, tag="v1")
            nc.sync.dma_start(
                v1p, v1_d[e0:e1 + 1].rearrange("two r h -> (two r) h"))
            u2p = p2u.tile([128, 2, HC, R], bf16, tag="u2")
            nc.sync.dma_start(
                u2p, u2r_d[e0:e1 + 1].rearrange("two p hc r -> p two hc r"))

            if c0 == 1:  # phase-3 loads, emitted late so they don't hog queues
                for cc in range(ERC):
                    nc.sync.dma_start(V2sb[cc], v2c_d[cc * 128:(cc + 1) * 128, :])


            # separate PSUM banks per col-tile half (start=True is per-bank)
            pt2a = ps_t2.tile([128, NT], f32, tag="t2a", name="pt2a")
            pt2b = ps_t2.tile([128, NT], f32, tag="t2b", name="pt2b")
            for hc in range(HC):
                ph0 = ps_h0.tile([128, NT], f32, tag="h0", name="ph0")
                ph1 = ps_h1.tile([128, NT], f32, tag="h1", name="ph1")
                # m2: row-tiled concurrent K=64 matmuls (one per expert)
                nc.tensor.matmul(ph0,
                                 lhsT=v1p[0:64, hc * 128:(hc + 1) * 128],
                                 rhs=T1Tp[c][0:64, :],
                                 start=True, stop=True)
                nc.tensor.matmul(ph1,
                                 lhsT=v1p[64:128, hc * 128:(hc + 1) * 128],
                                 rhs=T1Tp[c][64:128, :],
                                 start=True, stop=True)
                hT0 = p2h.tile([128, NT], bf16, tag="h0", name="hT0")
                hT1 = p2h.tile([128, NT], bf16, tag="h1", name="hT1")
                # hT0 on ACT, hT1 on DVE every hc (parallel lanes); every
                # ~5th hT1 flips to ACT so the faster engine carries ~59%
                relu_evac(hT0, ph0, b1r_sb[:, e0 * HC + hc:e0 * HC + hc + 1],
                          on_act=True)
                relu_evac(hT1, ph1, b1r_sb[:, e1 * HC + hc:e1 * HC + hc + 1],
                          on_act=(hc % 16 in (2, 7, 12)))
                # m3: col-tiled concurrent pair, bf16, accumulate over hc
                nc.tensor.matmul(pt2a[0:64, :], lhsT=u2p[:, 0, hc, :], rhs=hT0,
                                 start=(hc == 0), stop=(hc == HC - 1),
                                 skip_group_check=True)
                nc.tensor.matmul(pt2b[64:128, :], lhsT=u2p[:, 1, hc, :], rhs=hT1,
                                 start=(hc == 0), stop=(hc == HC - 1),
                                 skip_group_check=True)
            nc.vector.tensor_tensor(T2Ts[c][0:64, :], pt2a[0:64, :],
                                    Wbc[c][0:64, :], op=ALU.mult)
            nc.vector.tensor_tensor(T2Ts[c][64:128, :], pt2b[64:128, :],
                                    Wbc[c][64:128, :], op=ALU.mult)

        s2.close()

        # ---------------- Phase 3: combine ----------------
        ps_o = ctx.enter_context(tc.tile_pool(name="ps_o", bufs=2, space="PSUM"))
        p3o = ctx.enter_context(tc.tile_pool(name="p3o", bufs=4))
        for t in range(TOKC):
            for dd in range(DD):
                po = ps_o.tile([128, 512], f32, tag="o", name="po")
                for c in range(ERC):
                    nc.tensor.matmul(po,
                                     lhsT=T2Ts[c][:, t * 128:(t + 1) * 128],
                                     rhs=V2sb[c][:, dd * 512:(dd + 1) * 512],
                                     start=(c == 0), stop=False)
                nc.tensor.matmul(po, lhsT=wT[:, t * 128:(t + 1) * 128],
                                 rhs=b2_sb[:, dd * 512:(dd + 1) * 512],
                                 start=False, stop=True)
                ob = p3o.tile([128, 512], f32, tag="ob")
                if (t * DD + dd) % 2 == 0:
                    nc.scalar.copy(ob, po)
                else:
                    nc.vector.tensor_copy(ob, po)
                nc.sync.dma_start(
                    out_d[t * 128:(t + 1) * 128, dd * 512:(dd + 1) * 512], ob)

    nc.compile()
    return nc


def build_tiny():
    """Trivial NEFF for measuring dispatch overhead."""
    from contextlib import ExitStack as _ES
    nc = bacc.Bacc("TRN2", debug=False)
    a_d = nc.dram_tensor("a", [1, 8], dt.float32, kind="ExternalInput").ap()
    o_d = nc.dram_tensor("o", [1, 8], dt.float32, kind="ExternalOutput").ap()
    with _ES() as ctx:
        tc = ctx.enter_context(tile.TileContext(nc))
        p = ctx.enter_context(tc.tile_pool(name="p", bufs=1))
        t = p.tile([1, 8], dt.float32, tag="t")
        nc.sync.dma_start(t, a_d)
        nc.sync.dma_start(o_d, t)
    nc.compile()
    return nc


def prep_inputs(x, u1, v1, b1, u2, v2, b2, gate_w, gate_b, cfg):
    """Host-side layout prep (pure reshapes/transposes) + per-core sharding."""
    E, D, H, R, NT = cfg["E"], cfg["D"], cfg["H"], cfg["R"], cfg["NT"]
    HC = H // 128
    ER = E * R
    ERC = E // 2
    f = lambda a: np.ascontiguousarray(np.asarray(a, dtype=np.float32))

    import ml_dtypes
    x = f(x)
    u1c = f(np.asarray(u1, np.float32).transpose(1, 0, 2).reshape(D, ER))
    v1 = np.ascontiguousarray(np.asarray(v1, np.float32).astype(ml_dtypes.bfloat16))
    # b1r[p, e*HC+hc] = b1[e, hc*128+p]  (per-partition bias columns)
    b1r = f(np.asarray(b1, np.float32).reshape(E, HC, 128).transpose(2, 0, 1)
            .reshape(128, E * HC))
    # u2r[e, p, hc, r] = u2[e, hc*128+p, r]
    u2r = np.ascontiguousarray(
        np.asarray(u2, np.float32).reshape(E, HC, 128, R).transpose(0, 2, 1, 3)
        .astype(ml_dtypes.bfloat16))
    v2c = f(np.asarray(v2, np.float32).reshape(ER, D))
    b2 = f(b2)
    gw = f(gate_w)
    gb = f(np.asarray(gate_b, np.float32).reshape(1, E))
    ident = np.eye(128, dtype=np.float32)
    onesr = np.ones((1, 128), dtype=np.float32)
    sel = np.zeros((E, ERC, 128), dtype=np.float32)
    for c in range(ERC):
        sel[2 * c, c, 0:64] = 1.0
        sel[2 * c + 1, c, 64:128] = 1.0

    ncores = x.shape[0] // NT
    shared = dict(u1c=u1c, v1=v1, b1r=b1r, u2r=u2r, v2c=v2c, b2=b2,
                  gw=gw, gb=gb, ident=ident, sel=sel, onesr=onesr)
    in_maps = []
    for c in range(ncores):
        m = dict(shared)
        m["x"] = np.ascontiguousarray(x[c * NT:(c + 1) * NT])
        in_maps.append(m)
    return in_maps


_BUILT = {}


def _get_nc(cfg_key=None):
    if cfg_key is None:
        cfg_key = "full"
    if cfg_key not in _BUILT:
        _BUILT[cfg_key] = build(FULL_CFG)
    return _BUILT[cfg_key]


def run(inputs, trace=False):
    """Run on 8 cores; returns (full_output, BassKernelResults)."""
    import concourse.bass_utils as bass_utils

    nc = _get_nc()
    in_maps = prep_inputs(**inputs, cfg=FULL_CFG)
    res = bass_utils.run_bass_kernel_spmd(
        nc, in_maps, core_ids=list(range(len(in_maps))), trace=trace)
    out = np.concatenate([r["out"] for r in res.results], axis=0)
    return out, res


def kernel(**inputs) -> np.ndarray:
    out, _ = run(inputs, trace=False)
    return out


if __name__ == "__main__":
    # smoke-build only
    nc = _get_nc()
    print("built ok:", nc)



# revision 23
# speedup vs baseline: 1.7117x; 1.7117x over previous
"""Trainium2 Bass kernel for nn_MixtureOfRanksLayer (moe_routing).

Two-launch expert-parallel design (device does ALL the math; the host only
moves data between launches — slicing, gathering rows by the device-computed
routing, and summing per-expert partial outputs, i.e. the unshard step):

LAUNCH 1 — data-parallel over tokens (512/core on 8 cores):
  T1  = x @ u1           [512, E*R=512]   rank projections for all experts
  lg  = x @ gate_w.T+b   [512, 8]         gate logits (same lhsT tiles)
  w   = top2-renormalized weights (masked-max + sigmoid(l1-l2), exact,
        identical math to the softmax-top2-renorm reference)
  outputs: T1 (bf16), w (f32, nonzero exactly at the top-2 experts)

HOST between launches: from w>0 derive each expert's token list; gather the
64 T1 columns of expert e at its tokens into a [cap, 64] block (cap =
per-expert capacity rounded to 384-slot chunks, computed from the actual
counts so it never overflows); lay out chunk c on SBUF partition half c%2
so the rank-64 matmuls of consecutive chunks run CONCURRENTLY on the two
row/column halves of the PE array (measured: tiled pairs stream at full
aggregate rate, one 384-row pair costs one stream).

LAUNCH 2 — expert-parallel (core e owns expert e, weights are NOT
replicated: 3.9MB/core instead of 21MB, the DMA roofline win):
  h   = relu(v1.T @ T1g + b1)   per hc chunk, evac ACT/DVE alternating
  T2  = u2.T @ h                 accumulated over hc (col-tiled halves)
  y   = (T2.T @ v2 + b2) * w     w applied as per-partition scale at evac
  output: y [cap, 2048] bf16

HOST: out[idx_e] += y_e[:count]  (the expert-parallel unshard/combine).

Measured HW facts this design is built on (microbench, 8xNC-v3):
  bf16 matmul ~0.345ns/row; fp32r is 2.8x SLOWER (so everything is bf16);
  row/col-tiled K=64/M=64 pairs at different tile positions stream
  concurrently; per-core DMA ~180B/ns; ACT/DVE evac ~1 col/cycle (PSUM
  source blocks DVE 2x modes) which makes h-evacuation the main non-PE
  cost: minimized by capacity (~1152 slots vs 4096 dense tokens/expert).
"""

from contextlib import ExitStack, nullcontext

import ml_dtypes
import numpy as np

import concourse.bacc as bacc
import concourse.mybir as mybir
import concourse.tile as tile

dt = mybir.dt
AF = mybir.ActivationFunctionType
ALU = mybir.AluOpType
AX = mybir.AxisListType

E, D, H, R = 8, 2048, 8192, 64
N_TOK = 4096
NCORES = 8
NT = N_TOK // NCORES     # launch-1 tokens per core
DC = D // 128            # contraction chunks over d_model
ER = E * R               # stacked expert-rank axis
HC = H // 128            # hidden chunks
SC = 384                 # launch-2 slot chunk (psum: 384 f32 = 1.5KB/bank)
TC = NT // 128           # launch-1 token chunks

bf16 = ml_dtypes.bfloat16


# --------------------------------------------------------------------------
# Launch 1: T1 + routing weights, data-parallel
# --------------------------------------------------------------------------
def build_l1(rep=0):
    f32 = dt.float32
    bf = dt.bfloat16
    nc = bacc.Bacc("TRN2", debug=False)

    xt_d = nc.dram_tensor("xt", [128, DC * NT], bf, kind="ExternalInput").ap()
    xl_d = nc.dram_tensor("xtl", [128, DC * NT], bf, kind="ExternalInput").ap()
    u1_d = nc.dram_tensor("u1c", [128, DC * ER], bf, kind="ExternalInput").ap()
    gw_d = nc.dram_tensor("gwt", [128, DC * 2 * E], bf, kind="ExternalInput").ap()
    gb_d = nc.dram_tensor("gbb", [128, E], f32, kind="ExternalInput").ap()
    t1_d = nc.dram_tensor("t1", [NT, ER], bf, kind="ExternalOutput").ap()
    w_d = nc.dram_tensor("w", [NT, E], f32, kind="ExternalOutput").ap()

    with ExitStack() as ctx:
        tc = ctx.enter_context(tile.TileContext(nc))
        const = ctx.enter_context(tc.tile_pool(name="const", bufs=1))
        sm = ctx.enter_context(tc.tile_pool(name="sm", bufs=2))
        outp = ctx.enter_context(tc.tile_pool(name="outp", bufs=2))
        ps_t1 = ctx.enter_context(tc.tile_pool(name="ps_t1", bufs=2, space="PSUM"))
        ps_lg = ctx.enter_context(tc.tile_pool(name="ps_lg", bufs=2, space="PSUM"))

        loop = tc.For_i(0, rep) if rep else nullcontext()
        with loop:
            xt = const.tile([128, DC, NT], bf, tag="xt")
            nc.sync.dma_start(xt, xt_d)
            xtl = const.tile([128, DC, NT], bf, tag="xtl")
            nc.sync.dma_start(xtl, xl_d)
            u1 = const.tile([128, DC, ER], bf, tag="u1")
            nc.sync.dma_start(u1, u1_d)
            gw = const.tile([128, DC, 2 * E], bf, tag="gw")
            nc.sync.dma_start(gw, gw_d)
            gbb = const.tile([128, E], f32, tag="gbb")
            nc.sync.dma_start(gbb, gb_d)

            for t in range(TC):
                pt = ps_t1.tile([128, ER], f32, tag="pt")
                pl = ps_lg.tile([128, 2 * E], f32, tag="pl")
                pl3 = ps_lg.tile([128, E], f32, tag="pl3")
                for c in range(DC):
                    lhs = xt[:, c, t * 128:(t + 1) * 128]
                    nc.tensor.matmul(pt, lhsT=lhs, rhs=u1[:, c, :],
                                     start=(c == 0), stop=(c == DC - 1),
                                     skip_group_check=True)
                    # logits in split-bf16: xhi@[gwhi|gwlo] and xlo@gwhi
                    nc.tensor.matmul(pl, lhsT=lhs, rhs=gw[:, c, :],
                                     start=(c == 0), stop=(c == DC - 1),
                                     skip_group_check=True)
                    nc.tensor.matmul(pl3,
                                     lhsT=xtl[:, c, t * 128:(t + 1) * 128],
                                     rhs=gw[:, c, 0:E],
                                     start=(c == 0), stop=(c == DC - 1),
                                     skip_group_check=True)
                t1o = outp.tile([128, ER], bf, tag="t1o")
                nc.vector.tensor_copy(t1o, pt)
                nc.sync.dma_start(t1_d[t * 128:(t + 1) * 128, :], t1o)

                # routing: exact top-2 renormalized softmax weights
                lgs = sm.tile([128, 2 * E], f32, tag="lgs")
                nc.vector.tensor_copy(lgs, pl)
                lg1 = sm.tile([128, E], f32, tag="lg1")
                nc.vector.tensor_tensor(lg1, pl3, gbb, op=ALU.add)
                lg0 = sm.tile([128, E], f32, tag="lg0")
                nc.vector.tensor_tensor(lg0, lgs[:, 0:E], lgs[:, E:2 * E],
                                        op=ALU.add)
                lg = sm.tile([128, E], f32, tag="lg")
                nc.vector.tensor_add(lg, lg0, lg1)
                l1 = sm.tile([128, 1], f32, tag="l1")
                nc.vector.reduce_max(out=l1, in_=lg, axis=AX.X)
                m1t = sm.tile([128, E], f32, tag="m1t")
                nc.vector.tensor_scalar(m1t, lg, l1, None, op0=ALU.is_equal)
                lm = sm.tile([128, E], f32, tag="lm")
                nc.vector.tensor_scalar(lm, m1t, -1e30, None, op0=ALU.mult)
                nc.vector.tensor_add(lm, lm, lg)
                l2 = sm.tile([128, 1], f32, tag="l2")
                nc.vector.reduce_max(out=l2, in_=lm, axis=AX.X)
                m2t = sm.tile([128, E], f32, tag="m2t")
                nc.vector.tensor_scalar(m2t, lm, l2, None, op0=ALU.is_equal)
                dif = sm.tile([128, 1], f32, tag="dif")
                nc.vector.tensor_sub(dif, l1, l2)
                s1v = sm.tile([128, 1], f32, tag="s1v")
                nc.scalar.activation(s1v, dif, AF.Sigmoid)
                s0v = sm.tile([128, 1], f32, tag="s0v")
                nc.scalar.activation(s0v, dif, AF.Sigmoid, scale=-1.0)
                wa = sm.tile([128, E], f32, tag="wa")
                nc.vector.tensor_scalar(wa, m1t, s1v, None, op0=ALU.mult)
                wb_ = sm.tile([128, E], f32, tag="wb_")
                nc.vector.tensor_scalar(wb_, m2t, s0v, None, op0=ALU.mult)
                wt = outp.tile([128, E], f32, tag="wt")
                nc.vector.tensor_add(wt, wa, wb_)
                nc.sync.dma_start(w_d[t * 128:(t + 1) * 128, :], wt)

    nc.compile()
    return nc


# --------------------------------------------------------------------------
# Launch 2: per-expert FFN over gathered slots, expert-parallel
# --------------------------------------------------------------------------
def build_l2(nch, rep=0):
    f32 = dt.float32
    bf = dt.bfloat16
    npair = (nch + 1) // 2
    cap = nch * SC
    nsub = nch * (SC // 128)
    DD = D // 512
    nc = bacc.Bacc("TRN2", debug=False)

    t1g_d = nc.dram_tensor("t1g", [128, npair * SC], bf, kind="ExternalInput").ap()
    v1_d = nc.dram_tensor("v1d", [128, H], bf, kind="ExternalInput").ap()
    u2_d = nc.dram_tensor("u2l", [128, HC * R], bf, kind="ExternalInput").ap()
    v2_d = nc.dram_tensor("v2d", [128, D], bf, kind="ExternalInput").ap()
    b1_d = nc.dram_tensor("b1l", [128, HC], f32, kind="ExternalInput").ap()
    b2_d = nc.dram_tensor("b2r", [1, D], bf, kind="ExternalInput").ap()
    wg_d = nc.dram_tensor("wgc", [128, nsub], f32, kind="ExternalInput").ap()
    on_d = nc.dram_tensor("onesk", [1, 128], bf, kind="ExternalInput").ap()
    y_d = nc.dram_tensor("y", [cap, D], bf, kind="ExternalOutput").ap()

    # chunk -> (pair tile index, partition half)
    halves = [(ch // 2, (ch % 2) * 64) for ch in range(nch)]

    with ExitStack() as ctx:
        tc = ctx.enter_context(tile.TileContext(nc))
        const = ctx.enter_context(tc.tile_pool(name="const", bufs=1))
        t2p = ctx.enter_context(tc.tile_pool(name="t2p", bufs=1))
        hsb = ctx.enter_context(tc.tile_pool(name="hsb", bufs=6))
        ysb = ctx.enter_context(tc.tile_pool(name="ysb", bufs=4))

        loop = tc.For_i(0, rep) if rep else nullcontext()
        with loop:
            t1g = const.tile([128, npair, SC], bf, tag="t1g")
            nc.sync.dma_start(t1g, t1g_d)
            v1d = const.tile([128, H], bf, tag="v1d")
            nc.sync.dma_start(v1d, v1_d)
            u2l = const.tile([128, HC, R], bf, tag="u2l")
            nc.sync.dma_start(u2l, u2_d)
            u2l2 = const.tile([128, HC, R], bf, tag="u2l2")
            nc.sync.dma_start(u2l2, u2_d)
            v2d = const.tile([128, D], bf, tag="v2d")
            nc.sync.dma_start(v2d, v2_d)
            b1l = const.tile([128, HC], f32, tag="b1l")
            nc.sync.dma_start(b1l, b1_d)
            b2r = const.tile([1, D], bf, tag="b2r")
            nc.sync.dma_start(b2r, b2_d)
            wgc = const.tile([128, nsub], f32, tag="wgc")
            nc.sync.dma_start(wgc, wg_d)
            onesk = const.tile([1, 128], bf, tag="onesk")
            nc.sync.dma_start(onesk, on_d)

            t2sb = t2p.tile([128, npair, SC], bf, tag="t2sb")

            # ---- phase A: h = relu(v1.T @ t1g + b1); T2 += u2.T @ h ----
            with ExitStack() as sA:
                ps_ha = sA.enter_context(
                    tc.tile_pool(name="ps_ha", bufs=3, space="PSUM"))
                ps_hb = sA.enter_context(
                    tc.tile_pool(name="ps_hb", bufs=3, space="PSUM"))
                ps_t2 = sA.enter_context(
                    tc.tile_pool(name="ps_t2", bufs=1, space="PSUM"))

                # software pipeline: m2+evac for hc runs DEPTH chunks ahead
                # of m3(hc), so the in-order PE never blocks on an evac it
                # just scheduled (evac ~525ns+2 sems vs ~250ns of PE work/hc)
                DEPTH = 3
                for pj in range(npair):
                    has_b = 2 * pj + 1 < nch
                    # separate PSUM banks per column half (same-bank col-tiled
                    # accumulation serializes the PE; separate banks stream)
                    pt2a = ps_t2.tile([128, SC], f32, tag="pt2a", name="pt2a")
                    pt2b = None
                    if has_b:
                        pt2b = ps_t2.tile([128, SC], f32, tag="pt2b", name="pt2b")
                    hq = {}
                    for step in range(HC + DEPTH):
                        if step < HC:
                            hc = step
                            hs = slice(hc * 128, (hc + 1) * 128)
                            pha = ps_ha.tile([128, SC], f32, tag="ha", name="ha")
                            nc.tensor.matmul(pha, lhsT=v1d[0:64, hs],
                                             rhs=t1g[0:64, pj, :],
                                             start=True, stop=True)
                            if has_b:
                                phb = ps_hb.tile([128, SC], f32, tag="hb",
                                                 name="hb")
                                nc.tensor.matmul(phb, lhsT=v1d[64:128, hs],
                                                 rhs=t1g[64:128, pj, :],
                                                 start=True, stop=True)
                            ha = hsb.tile([128, SC], bf, tag="ha", name="sha")
                            bias = b1l[:, hc:hc + 1]
                            if hc % 2 == 0:
                                nc.scalar.activation(ha, pha, AF.Relu, bias=bias)
                            else:
                                nc.vector.tensor_scalar(ha, pha, bias, 0.0,
                                                        op0=ALU.add, op1=ALU.max)
                            hb = None
                            if has_b:
                                hb = hsb.tile([128, SC], bf, tag="hb", name="shb")
                                if hc % 2 == 0:
                                    nc.vector.tensor_scalar(hb, phb, bias, 0.0,
                                                            op0=ALU.add,
                                                            op1=ALU.max)
                                else:
                                    nc.scalar.activation(hb, phb, AF.Relu,
                                                         bias=bias)
                            hq[hc] = (ha, hb)
                        mc = step - DEPTH
                        if mc >= 0:
                            ha, hb = hq.pop(mc)
                            nc.tensor.matmul(pt2a[0:64, :], lhsT=u2l[:, mc, :],
                                             rhs=ha,
                                             start=(mc == 0), stop=(mc == HC - 1),
                                             skip_group_check=True)
                            if hb is not None:
                                nc.tensor.matmul(pt2b[64:128, :],
                                                 lhsT=u2l2[:, mc, :], rhs=hb,
                                                 start=(mc == 0),
                                                 stop=(mc == HC - 1),
                                                 skip_group_check=True)
                    nc.vector.tensor_copy(t2sb[0:64, pj, :], pt2a[0:64, :])
                    if has_b:
                        nc.vector.tensor_copy(t2sb[64:128, pj, :],
                                              pt2b[64:128, :])

            # ---- phase B: y = (T2.T @ v2 + b2) * w ----
            sB = ExitStack()
            ps_y = sB.enter_context(
                tc.tile_pool(name="ps_y", bufs=4, space="PSUM"))
            groups = [tuple(c for c in (2 * i, 2 * i + 1) if c < nch)
                      for i in range((nch + 1) // 2)]
            for grp in groups:
                for s3 in range(SC // 128):
                    for dd in range(DD):
                        pys = []
                        for ch in grp:
                            pj, half = halves[ch]
                            py = ps_y.tile([128, 512], f32, tag="py", name="py")
                            nc.tensor.matmul(
                                py,
                                lhsT=t2sb[half:half + 64, pj,
                                          s3 * 128:(s3 + 1) * 128],
                                rhs=v2d[half:half + 64, dd * 512:(dd + 1) * 512],
                                start=True, stop=False, skip_group_check=True)
                            pys.append(py)
                        for ch, py in zip(grp, pys):
                            nc.tensor.matmul(py, lhsT=onesk,
                                             rhs=b2r[0:1, dd * 512:(dd + 1) * 512],
                                             start=False, stop=True,
                                             skip_group_check=True)
                        for k, (ch, py) in enumerate(zip(grp, pys)):
                            sub = ch * (SC // 128) + s3
                            yo = ysb.tile([128, 512], bf, tag="yo", name="yo")
                            if (sub * DD + dd) % 2 == 0:
                                nc.scalar.activation(yo, py, AF.Copy,
                                                     scale=wgc[:, sub:sub + 1])
                            else:
                                nc.vector.tensor_scalar(yo, py,
                                                        wgc[:, sub:sub + 1],
                                                        None, op0=ALU.mult)
                            nc.sync.dma_start(
                                y_d[sub * 128:(sub + 1) * 128,
                                    dd * 512:(dd + 1) * 512], yo)
            sB.close()

    nc.compile()
    return nc


# --------------------------------------------------------------------------
# Host-side prep / orchestration
# --------------------------------------------------------------------------
_BUILT = {}


def _get(key, builder):
    if key not in _BUILT:
        _BUILT[key] = builder()
    return _BUILT[key]


def prep_l1(x, u1, gate_w, gate_b):
    x = np.asarray(x, np.float32)
    xb = x.astype(bf16)
    xlo = (x - xb.astype(np.float32)).astype(bf16)
    u1c = (np.asarray(u1, np.float32).transpose(1, 0, 2).reshape(D, ER)
           .astype(bf16))
    u1L = np.ascontiguousarray(
        u1c.reshape(DC, 128, ER).transpose(1, 0, 2)).reshape(128, DC * ER)
    gwf = np.asarray(gate_w, np.float32).T          # [D, E]
    gwhi = gwf.astype(bf16)
    gwlo = (gwf - gwhi.astype(np.float32)).astype(bf16)
    gwcat = np.concatenate(
        [gwhi.reshape(DC, 128, E), gwlo.reshape(DC, 128, E)], axis=2)
    gwL = np.ascontiguousarray(
        gwcat.transpose(1, 0, 2)).reshape(128, DC * 2 * E)
    gbb = np.ascontiguousarray(np.broadcast_to(
        np.asarray(gate_b, np.float32).reshape(1, E), (128, E)))
    maps = []
    for c in range(NCORES):
        def lay(a):
            s = np.ascontiguousarray(a[c * NT:(c + 1) * NT].T)  # [D, NT]
            return np.ascontiguousarray(
                s.reshape(DC, 128, NT).transpose(1, 0, 2)).reshape(128, DC * NT)
        maps.append(dict(xt=lay(xb), xtl=lay(xlo), u1c=u1L, gwt=gwL, gbb=gbb))
    return maps


def route(w_full):
    """Token lists per expert from the device-computed weights."""
    idxs, wgs = [], []
    for e in range(E):
        idx = np.nonzero(w_full[:, e] > 0)[0]
        idxs.append(idx)
        wgs.append(w_full[idx, e])
    maxc = max(len(i) for i in idxs)
    nch = max(2, -(-maxc // SC))
    return idxs, wgs, nch


def prep_l2(t1_full, idxs, wgs, nch, v1, b1, u2, v2, b2):
    npair = (nch + 1) // 2
    cap = nch * SC
    nsub = nch * (SC // 128)
    v1 = np.asarray(v1, np.float32)
    u2 = np.asarray(u2, np.float32)
    v2 = np.asarray(v2, np.float32)
    b1 = np.asarray(b1, np.float32)
    b2 = np.asarray(b2, np.float32)
    onesk = np.ones((1, 128), dtype=bf16)
    maps = []
    for e in range(E):
        idx, wg = idxs[e], wgs[e]
        pad = np.zeros((cap, R), dtype=bf16)
        pad[:len(idx)] = t1_full[idx, e * R:(e + 1) * R]
        arr = pad.reshape(nch, SC, R).transpose(0, 2, 1)  # [nch, R, SC]
        t1g = np.zeros((128, npair, SC), dtype=bf16)
        for ch in range(nch):
            t1g[(ch % 2) * 64:(ch % 2) * 64 + 64, ch // 2, :] = arr[ch]
        wp = np.zeros((cap,), np.float32)
        wp[:len(idx)] = wg
        maps.append(dict(
            t1g=t1g.reshape(128, npair * SC),
            v1d=np.concatenate([v1[e], v1[e]], 0).astype(bf16),
            u2l=np.ascontiguousarray(
                u2[e].reshape(HC, 128, R).transpose(1, 0, 2)
            ).reshape(128, HC * R).astype(bf16),
            v2d=np.concatenate([v2[e], v2[e]], 0).astype(bf16),
            b1l=np.ascontiguousarray(b1[e].reshape(HC, 128).T),
            b2r=b2[e].reshape(1, D).astype(bf16),
            wgc=np.ascontiguousarray(wp.reshape(nsub, 128).T),
            onesk=onesk,
        ))
    return maps


def run(inputs, return_info=False):
    import concourse.bass_utils as bass_utils

    x = np.asarray(inputs["x"], np.float32)
    l1_maps = prep_l1(x, inputs["u1"], inputs["gate_w"], inputs["gate_b"])
    nc1 = _get(("l1", 0), lambda: build_l1(0))
    res1 = bass_utils.run_bass_kernel_spmd(
        nc1, l1_maps, core_ids=list(range(NCORES)))
    t1_full = np.concatenate([r["t1"] for r in res1.results], axis=0)
    w_full = np.concatenate(
        [r["w"] for r in res1.results], axis=0).astype(np.float32)

    idxs, wgs, nch = route(w_full)
    l2_maps = prep_l2(t1_full, idxs, wgs, nch,
                      inputs["v1"], inputs["b1"], inputs["u2"],
                      inputs["v2"], inputs["b2"])
    nc2 = _get(("l2", nch, 0), lambda: build_l2(nch, 0))
    res2 = bass_utils.run_bass_kernel_spmd(
        nc2, l2_maps, core_ids=list(range(NCORES)))

    out = np.zeros((N_TOK, D), np.float32)
    for e in range(E):
        ye = np.asarray(res2.results[e]["y"])[:len(idxs[e])]
        out[idxs[e]] += ye.astype(np.float32)
    if return_info:
        return out, dict(l1_maps=l1_maps, l2_maps=l2_maps, nch=nch)
    return out


def kernel(**inputs) -> np.ndarray:
    return run(inputs)


if __name__ == "__main__":
    nc1 = build_l1(0)
    nc2 = build_l2(3, 0)
    print("built ok")
